# revision 1
# baseline (speedup 1.0000x reference)
"""Fused multi-head attention (QKV + RoPE2D + softmax + out-proj) on 8 TRN2 cores.

Sharding: batch-parallel. B == n_cores == 8, so each core runs one batch
element end-to-end; weights are replicated. No collectives needed.

Per-core dataflow (matmul operands in KDTYPE, accumulation in f32 PSUM):
  phase 1: v[n,dd]   = x @ w_v.T         (lhsT = xT tiles, rhs = w_vT tiles)
           qkT[D,n]  = (x @ w_{q,k}.T).T (lhsT = w chunks,  rhs = xT tiles)
           RoPE on qkT: rot = PERM @ qkT (matmul), then
           qk' = qkT*cos2 + rot*sin2   (signs folded into sin2 host-side)
  phase 2: per head h:
           sT[j,i] = k'_h-tiles.T @ q'_h  (K=64)
           e = exp(sT/8) on ACT, PSUM->SBUF
           av = [v_h | ones].T @ e accumulated over j-tiles ->
                rows 0:64 = unnormalized out.T, row 64 = softmax sums
           outT rows for h = av[0:64]; sums collected, then per 8-head batch:
           reciprocal + K=8 selection-matmul broadcast + multiply
  phase 3: y[n,e] = outT-tiles.T @ w_projT + bias -> DMA out

The next pair's QKV/RoPE matmuls are interleaved into the current pair's
attention emission (generator-based software pipelining) so the in-order
PE queue always has ready work during exp waits.
"""

import os
import numpy as np

B, N, C = 8, 1024, 1024
H, HD = 16, 64
P = 128
NT = N // P          # 8 n-tiles
CT = C // P          # 8 c-tiles
TP = H // 2          # 8 head-pairs (qk D-tiles per q/k)
SCALE = HD ** -0.5   # 1/8

KDTYPE = os.environ.get("BASS_ATTN_DTYPE", "bfloat16")

_CACHE = {}


def _build_nc():
    import concourse.mybir as mybir
    from concourse import bacc, tile
    from contextlib import ExitStack

    f32 = mybir.dt.float32
    mdt = getattr(mybir.dt, KDTYPE)

    nc = bacc.Bacc(
        "TRN2", target_bir_lowering=False, debug=False,
        enable_asserts=False, num_devices=B,
    )

    xT_d = nc.dram_tensor("xT", [C, N], mdt, kind="ExternalInput")
    cos2_d = nc.dram_tensor("cos2", [P, N], f32, kind="ExternalInput")
    sin2_d = nc.dram_tensor("sin2", [P, N], f32, kind="ExternalInput")
    perm_d = nc.dram_tensor("perm", [P, P], mdt, kind="ExternalInput")
    wq_d = nc.dram_tensor("wq", [TP, CT, P, P], mdt, kind="ExternalInput")
    wk_d = nc.dram_tensor("wk", [TP, CT, P, P], mdt, kind="ExternalInput")
    wv_d = nc.dram_tensor("wv", [CT, P, C], mdt, kind="ExternalInput")
    wp_d = nc.dram_tensor("wp", [CT, P, C], mdt, kind="ExternalInput")
    sel_d = nc.dram_tensor("sel", [TP, TP, P], mdt, kind="ExternalInput")
    bias_d = nc.dram_tensor("bias", [1, C], f32, kind="ExternalInput")
    out_d = nc.dram_tensor("out", [N, C], f32, kind="ExternalOutput")

    with tile.TileContext(nc) as tc, ExitStack() as ctx:
        const = ctx.enter_context(tc.tile_pool(name="const", bufs=1))
        vpool = ctx.enter_context(tc.tile_pool(name="vpool", bufs=1))
        otpool = ctx.enter_context(tc.tile_pool(name="otpool", bufs=1))
        qkpre = ctx.enter_context(tc.tile_pool(name="qkpre", bufs=2))
        qkfin = ctx.enter_context(tc.tile_pool(name="qkfin", bufs=6))
        expp = ctx.enter_context(tc.tile_pool(name="expp", bufs=3))
        rcpp = ctx.enter_context(tc.tile_pool(name="rcpp", bufs=2))
        tmpp = ctx.enter_context(tc.tile_pool(name="tmpp", bufs=2))
        sT_ps = ctx.enter_context(tc.tile_pool(name="sT_ps", bufs=2, space="PSUM"))
        av_ps = ctx.enter_context(tc.tile_pool(name="av_ps", bufs=1, space="PSUM"))
        mi_ps = ctx.enter_context(tc.tile_pool(name="mi_ps", bufs=2, space="PSUM"))

        # constants
        perm = const.tile([P, P], mdt)
        nc.sync.dma_start(perm[:], perm_d[:])
        cos2 = const.tile([P, N], f32)
        nc.sync.dma_start(cos2[:], cos2_d[:])
        sin2 = const.tile([P, N], f32)
        nc.sync.dma_start(sin2[:], sin2_d[:])
        bias_bc = const.tile([P, C], f32)
        nc.sync.dma_start(bias_bc[:1, :], bias_d[:])
        nc.gpsimd.partition_broadcast(bias_bc[:], bias_bc[:1, :])

        # v storage: [128 j-local, NT j-tiles, H heads x (64 v + 1 ones col)]
        v_all = vpool.tile([P, NT, H * (HD + 1)], mdt)
        ones_c = const.tile([P, H], f32)
        nc.vector.memset(ones_c[:], 1.0)
        for jt in range(NT):
            nc.vector.tensor_copy(
                v_all[:, jt, :].rearrange("p (h c) -> p h c", c=HD + 1)[:, :, HD:],
                ones_c[:].rearrange("p (h o) -> p h o", o=1))
        # out.T accumulator: [128 c-local, CT c-tiles, 1024 n]
        outT = otpool.tile([P, CT, N], mdt)
        # per-head softmax sums: two batches of 8 heads so the first
        # batch's normalization overlaps the second batch's attention
        sums_b = [otpool.tile([TP, N], f32, tag=f"sums{i}", name=f"sums{i}")
                  for i in range(2)]
        rcp_b = [otpool.tile([TP, N], f32, tag=f"rcpf{i}", name=f"rcpf{i}")
                 for i in range(2)]
        rcp16_b = [otpool.tile([TP, N], mdt, tag=f"rcp16{i}", name=f"rcp16{i}")
                   for i in range(2)]
        selc = const.tile([TP, TP, P], mdt)  # head-pair selection matrices
        nc.sync.dma_start(selc[:], sel_d[:].rearrange("a h p -> h a p"))

        with tc.tile_pool(name="xtp", bufs=1) as xtp, \
             tc.tile_pool(name="wvp", bufs=1) as wvp, \
             tc.tile_pool(name="wch", bufs=3) as wch:
            xt = xtp.tile([P, CT, N], mdt)       # xT tiles, ct-indexed
            wv = wvp.tile([P, CT, C], mdt)
            # split column-wise and interleave so the first v matmuls (which
            # touch only the leading columns) start as early as possible
            for cs in (slice(0, 512), slice(512, N)):
                for ct in range(CT):
                    nc.sync.dma_start(xt[:, ct, cs],
                                      xT_d[ct * P:(ct + 1) * P, cs])
                    nc.sync.dma_start(wv[:, ct, cs], wv_d[ct][:, cs])

            # ---- phase 1a: v = x @ w_v.T (straight orientation) ----
            for nt in range(NT):
                for ch in range(2):
                    vps = mi_ps.tile([P, 512], f32, tag="mi", name="vps")
                    for ct in range(CT):
                        nc.tensor.matmul(
                            vps[:],
                            xt[:, ct, nt * P:(nt + 1) * P],
                            wv[:, ct, ch * 512:(ch + 1) * 512],
                            start=(ct == 0), stop=(ct == CT - 1),
                        )
                    nc.vector.tensor_copy(
                        v_all[:, nt, :].rearrange(
                            "p (h c) -> p h c", c=HD + 1)[:, 8 * ch:8 * ch + 8, :HD],
                        vps[:])

            # ---- phase 1b+2: software-pipelined head-pairs ----
            qk_fin = {}

            def prepare_pair(t):
                """Generator: qkv D-tiles + RoPE for pair t, yielding after
                each PE instruction so it can interleave into attention."""
                qk_tiles = []
                for wsrc in (wq_d, wk_d):
                    pre = qkpre.tile([P, N], mdt, tag="pre", name="pre")
                    wcht = wch.tile([P, CT, P], mdt, tag="w", name="wcht")
                    nc.sync.dma_start(
                        wcht[:], wsrc[t].rearrange("a p c -> p a c"))
                    for ch in range(2):
                        qps = mi_ps.tile([P, 512], f32, tag="mi", name="qps")
                        for ct in range(CT):
                            nc.tensor.matmul(
                                qps[:],
                                wcht[:, ct, :],
                                xt[:, ct, ch * 512:(ch + 1) * 512],
                                start=(ct == 0), stop=(ct == CT - 1),
                            )
                            yield
                        nc.vector.tensor_copy(
                            pre[:, ch * 512:(ch + 1) * 512], qps[:])
                    # RoPE: fin = pre*cos2 + (PERM @ pre)*sin2
                    fin = qkfin.tile([P, N], mdt, tag="fin", name="fin")
                    for ch in range(2):
                        sl = slice(ch * 512, (ch + 1) * 512)
                        rot = mi_ps.tile([P, 512], f32, tag="mi", name="rot")
                        nc.tensor.matmul(rot[:], perm[:], pre[:, sl],
                                         start=True, stop=True)
                        yield
                        tmp = tmpp.tile([P, 512], f32, tag="tmp", name="tmp")
                        nc.vector.tensor_mul(tmp[:], pre[:, sl], cos2[:, sl])
                        nc.vector.tensor_mul(fin[:, sl], rot[:], sin2[:, sl])
                        nc.vector.tensor_add(fin[:, sl], fin[:, sl], tmp[:])
                    qk_tiles.append(fin)
                qk_fin[t] = qk_tiles

            def pull(feeder, k):
                if feeder is None:
                    return None
                for _ in range(k):
                    if next(feeder, "done") == "done":
                        return None
                return feeder

            feeder = prepare_pair(0)
            pull(feeder, 10 ** 6)
            for t in range(TP):
                feeder = prepare_pair(t + 1) if t + 1 < TP else None
                qf, kf = qk_fin.pop(t)

                for hh in range(2):  # head = 2*t + hh
                    h = 2 * t + hh
                    ro = slice(64 * hh, 64 * hh + 64)
                    av = av_ps.tile([HD + 1, N], f32, tag="av", name="av")
                    for jt in range(NT):
                        sT = sT_ps.tile([P, N], f32, tag="sT", name="sT")
                        for ch in range(2):
                            nc.tensor.matmul(
                                sT[:, ch * 512:(ch + 1) * 512],
                                kf[ro, jt * P:(jt + 1) * P],
                                qf[ro, ch * 512:(ch + 1) * 512],
                                start=True, stop=True,
                            )
                        ex = expp.tile([P, N], mdt, tag="ex", name="ex")
                        nc.scalar.activation(
                            ex[:], sT[:],
                            mybir.ActivationFunctionType.Exp, scale=SCALE)
                        # stationary [K=128 j, M=65]: head h's v cols + ones
                        vh = v_all[:, jt, h * (HD + 1):(h + 1) * (HD + 1)]
                        for ch in range(2):
                            nc.tensor.matmul(
                                av[:, ch * 512:(ch + 1) * 512],
                                vh,
                                ex[:, ch * 512:(ch + 1) * 512],
                                start=(jt == 0), stop=(jt == NT - 1),
                            )
                        feeder = pull(feeder, 3)
                    # av rows 0:64 = unnormalized out.T, row 64 = softmax sums
                    nc.vector.tensor_copy(outT[ro, t, :], av[:HD, :])
                    # DVE outputs must start at partition 0; bounce the sums
                    # row through SBUF and DMA it onto partition h%8
                    cp1 = rcpp.tile([1, N], f32, tag="cp1", name="cp1")
                    nc.vector.tensor_copy(cp1[:], av[HD:HD + 1, :])
                    nc.sync.dma_start(sums_b[h // 8][h % 8:h % 8 + 1, :], cp1[:])
                pull(feeder, 10 ** 6)

                if t == 5 or t == TP - 1:
                    # normalize a batch of 8 heads: K=8 selection matmul
                    # broadcasts each head's reciprocal sum over its 64 outT
                    # rows; batch 0 is emitted two pairs late so its chain is
                    # off the critical path
                    g = 0 if t == 5 else 1
                    nc.vector.reciprocal_approx_fast(rcp_b[g][:], sums_b[g][:])
                    nc.vector.tensor_copy(rcp16_b[g][:], rcp_b[g][:])
                    for tt in range(4 * g, 4 * g + 4):
                        for ch in range(2):
                            sl = slice(ch * 512, (ch + 1) * 512)
                            rb = mi_ps.tile([P, 512], f32, tag="mi", name="rb")
                            nc.tensor.matmul(rb[:], selc[:, tt, :],
                                             rcp16_b[g][:, sl],
                                             start=True, stop=True)
                            nc.vector.tensor_mul(outT[:HD, tt, sl],
                                                 outT[:HD, tt, sl], rb[:HD, :])
                            nc.vector.tensor_mul(outT[HD:, tt, sl],
                                                 outT[HD:, tt, sl], rb[HD:, :])

        # ---- phase 3: y = outT.T @ w_projT + bias ----
        with tc.tile_pool(name="wpp", bufs=1) as wpp, \
             tc.tile_pool(name="ybp", bufs=3) as ybp:
            wp = wpp.tile([P, CT, C], mdt)
            for ct in range(CT):
                nc.sync.dma_start(wp[:, ct, :], wp_d[ct])
            for nt in range(NT):
                yps = sT_ps.tile([P, N], f32, tag="sT", name="yps")
                for ch in range(2):
                    sl = slice(ch * 512, (ch + 1) * 512)
                    for ct in range(CT):
                        nc.tensor.matmul(
                            yps[:, sl],
                            outT[:, ct, nt * P:(nt + 1) * P],
                            wp[:, ct, sl],
                            start=(ct == 0), stop=(ct == CT - 1),
                        )
                yb = ybp.tile([P, N], f32, tag="yb", name="yb")
                nc.vector.tensor_add(yb[:], yps[:], bias_bc[:])
                nc.sync.dma_start(out_d[nt * P:(nt + 1) * P, :], yb[:])

    nc.compile()
    return nc


def get_nc():
    if "nc" not in _CACHE:
        _CACHE["nc"] = _build_nc()
    return _CACHE["nc"]


def _host_inputs(x, xpos, w_qkv, w_proj, b_proj):
    """Host-side reshapes: transposes, RoPE tables, weight packing."""
    x = np.asarray(x, dtype=np.float32)
    xpos = np.asarray(xpos)
    w_qkv = np.asarray(w_qkv, dtype=np.float32)
    w_proj = np.asarray(w_proj, dtype=np.float32)
    b_proj = np.asarray(b_proj, dtype=np.float32).reshape(1, C)

    xT = np.ascontiguousarray(x.transpose(0, 2, 1))  # [B, C, N]

    # RoPE tables in [d, n] orientation, two head-copies stacked to 128 rows.
    inv_freq = (100.0 ** (-np.arange(16, dtype=np.float64) / 16.0))
    py = xpos[..., 0].astype(np.float64)  # [B, N]
    px = xpos[..., 1].astype(np.float64)
    angy = py[:, :, None] * inv_freq      # [B, N, 16]
    angx = px[:, :, None] * inv_freq
    cos64 = np.concatenate(
        [np.cos(angy), np.cos(angy), np.cos(angx), np.cos(angx)], axis=2)
    sin64 = np.concatenate(
        [-np.sin(angy), np.sin(angy), -np.sin(angx), np.sin(angx)], axis=2)
    cos2 = np.ascontiguousarray(
        np.tile(cos64, (1, 1, 2)).transpose(0, 2, 1)).astype(np.float32)
    sin2 = np.ascontiguousarray(
        np.tile(sin64, (1, 1, 2)).transpose(0, 2, 1)).astype(np.float32)

    # permutation matrix: sigma(d) = d XOR 16 within each 64-block
    r = np.arange(P)
    sig = (r // 64) * 64 + ((r % 64) ^ 16)
    perm = np.zeros((P, P), dtype=np.float32)
    perm[sig, r] = 1.0  # perm[k, m] = 1 iff k == sigma(m)

    wq = np.zeros((TP, CT, P, P), dtype=np.float32)
    wk = np.zeros((TP, CT, P, P), dtype=np.float32)
    for t in range(TP):
        for ct in range(CT):
            wq[t, ct] = w_qkv[t * P:(t + 1) * P, ct * P:(ct + 1) * P].T
            wk[t, ct] = w_qkv[C + t * P:C + (t + 1) * P, ct * P:(ct + 1) * P].T
    wv = np.ascontiguousarray(
        w_qkv[2 * C:3 * C, :].T.reshape(CT, P, C))   # [ct][c-local, dd]
    wp = np.ascontiguousarray(w_proj.T.reshape(CT, P, C))  # [ct][c-local, e]

    sel = np.zeros((TP, TP, P), dtype=np.float32)
    for t in range(TP):
        sel[t, 2 * (t % 4), :HD] = 1.0
        sel[t, 2 * (t % 4) + 1, HD:] = 1.0

    if KDTYPE == "bfloat16":
        import ml_dtypes

        def mcast(a):
            return np.ascontiguousarray(a).astype(ml_dtypes.bfloat16)
    else:
        def mcast(a):
            return np.ascontiguousarray(a)

    shared = dict(perm=mcast(perm), wq=mcast(wq), wk=mcast(wk),
                  wv=mcast(wv), wp=mcast(wp), sel=mcast(sel), bias=b_proj)
    in_maps = []
    for b in range(B):
        m = dict(shared)
        m["xT"] = mcast(xT[b])
        m["cos2"] = cos2[b]
        m["sin2"] = sin2[b]
        in_maps.append(m)
    return in_maps


def kernel(x, xpos, w_qkv, w_proj, b_proj):
    from concourse import bass_utils

    nc = get_nc()
    in_maps = _host_inputs(x, xpos, w_qkv, w_proj, b_proj)
    res = bass_utils.run_bass_kernel_spmd(
        nc, in_maps, core_ids=list(range(B)),
        trace=bool(int(os.environ.get("BASS_ATTN_TRACE", "0"))),
    )
    out = np.stack([res.results[b]["out"] for b in range(B)], axis=0)
    _CACHE["last_results"] = res
    return out

